# Initial kernel scaffold
#
"""DistortionConvLayer Trainium2 kernel (8-core SPMD, Bass/Tile).

Math: the distortion offsets depend only on (h, tap) and are compile-time
constants. Per (h, tap) the bilinear sample rows y0/y1 are fixed rows and the
x-coordinate is w + s with a constant integer shift s and constant fractional
part. Folding the four bilinear corner weights into the conv kernel gives

    out[b,h] = relu( sum_j  G[h,j]^T @ R[h,j]  + bias )            (F x W)

where chunk j has a (row y, shift s) pair,
    R[h,j] = [ Xc[y, w+s] ; Xc[y, w+s+1] ]   (128 x W, c-major, circular x)
    G[h,j] = sum over taps (k, yrow) hitting (y, s):
                [ wy*wx0 * K_k ; wy*wx1 * K_k ]   (128 x F)

At most 14 chunks per h; G is built on-chip from the runtime kernel with 14
init + 8 accumulate DVE ops per h (alpha vectors are per-core constants).

Sharding: H is split into 8 blocks of 16 rows; each core processes all 4
batch images for its rows. Per-core tables (AP offsets, alpha vectors) ship
as extra input tensors; offsets are read into engine registers at runtime so
a single SPMD program serves all cores.
"""

import numpy as np

# problem dims (hardcoded per spec)
B, H, W, C, F = 4, 128, 256, 64, 128
KH = KW = 3
IN_H, IN_W = H + 2, W + 2
NCORE = 8
NH = H // NCORE            # h rows per core
NROW = NH + 6              # input rows per core: [h0-2, h0+NH+4)
MARG = 1                   # left margin in circular row layout
ROWQ = 260                 # stored row width: q in [0,260) holds circ col (q-1)
NCHUNK = 14                # padded chunks per h
NACC = 8                   # padded accumulate terms per h
NTERM_COLS = NCHUNK + NACC # alpha columns per h


# ---------------------------------------------------------------- host tables
def _make_offset(h, w, dilation=1.0, skydome=True):
    pi = np.pi
    unit_w = 2.0 * pi / w
    unit_h = pi / (2.0 * h) if skydome else pi / h
    rho = np.tan(unit_w) * dilation
    v = np.array([0.0, 1.0, 0.0])
    r_grid = np.array(
        [[1, -1], [1, 0], [1, 1], [0, -1], [0, 0], [0, 1], [-1, -1], [-1, 0], [-1, 1]],
        dtype=np.float64,
    )
    xc = int(w * 0.5)
    theta = (xc - 0.5 * w) * unit_w
    y = np.arange(h, dtype=np.float64)
    phi = (h - y) * unit_h if skydome else (h * 0.5 - y) * unit_h
    p_u = np.stack(
        [np.cos(phi) * np.cos(theta), np.sin(phi), np.cos(phi) * np.sin(theta)], axis=-1
    )
    t_x = np.cross(np.broadcast_to(v, p_u.shape), p_u)
    t_y = np.cross(p_u, t_x)
    r_sphere = rho * (
        r_grid[None, :, 0, None] * t_x[:, None, :]
        + r_grid[None, :, 1, None] * t_y[:, None, :]
    )
    p_ur = p_u[:, None, :] + r_sphere
    ux, uy, uz = p_ur[..., 0], p_ur[..., 1], p_ur[..., 2]
    base = np.arctan2(uz, ux)
    theta_r = np.where(
        ux > 0,
        base,
        np.where(
            ux < 0,
            np.where(uz >= 0, base + pi, base - pi),
            np.where(uz > 0, pi * 0.5, -pi * 0.5),
        ),
    )
    phi_r = np.arcsin(uy)
    x_r = (theta_r / pi + 1.0) * 0.5 * w
    y_r = (1.0 - 2.0 * phi_r / pi) * h if skydome else (0.5 - phi_r / pi) * h
    k = np.stack([x_r, y_r], axis=-1)
    off = k - k[:, 4:5, :]
    return off.astype(np.float32)  # [h, 9, 2]


def _build_chunk_tables():
    """Per-h chunk decomposition.

    Returns (chunks, terms): chunks[h] = [(y, s)], terms[h] = list of
    (tap k, chunk idx, a_top, a_bot) with 18 entries.
    """
    off = _make_offset(H, W)
    chunks_all, terms_all = [], []
    for h in range(H):
        ids, chunks, terms = {}, [], []
        for k in range(KH * KW):
            dy, dx = k // 3, k % 3
            cy, cx = np.float32(off[h, k, 0]), np.float32(off[h, k, 1])
            yv = float(np.float32(h + dy) + cy)
            yv = min(max(yv, 0.0), float(IN_H - 1))
            y0 = min(max(int(np.floor(yv)), 0), IN_H - 1)
            y1 = min(y0 + 1, IN_H - 1)
            wy0, wy1 = float(y1 - yv), float(yv - y0)
            s = dx + int(np.floor(cx))
            fx = float(dx + cx - np.floor(cx + dx))
            wx0, wx1 = 1.0 - fx, fx
            for yy, wy in ((y0, wy0), (y1, wy1)):
                key = (yy, s)
                if key not in ids:
                    ids[key] = len(chunks)
                    chunks.append(key)
                terms.append((k, ids[key], wy * wx0, wy * wx1))
        assert len(chunks) <= NCHUNK
        chunks_all.append(chunks)
        terms_all.append(terms)
    return chunks_all, terms_all


def _core_tables(core, chunks_all, terms_all):
    """Per-core constant tensors: alphas [128, NH*NTERM_COLS] and tbl int32
    [NH*(NCHUNK + NCHUNK + NACC + NACC)] = per t: [offs 14][kinit 14][kacc 8][tacc 8]."""
    h0 = core * NH
    alphas = np.zeros((128, NH * NTERM_COLS), np.float32)
    tbl = np.zeros((NH, NCHUNK * 2 + NACC * 2), np.int32)
    for t in range(NH):
        h = h0 + t
        chunks, terms = chunks_all[h], terms_all[h]
        # group terms by chunk; first term of each chunk -> init, rest -> accum
        by_chunk = [[] for _ in chunks]
        for (k, j, a_top, a_bot) in terms:
            by_chunk[j].append((k, a_top, a_bot))
        acc_list = []
        for j in range(NCHUNK):
            if j < len(chunks):
                y, s = chunks[j]
                r = y - (h0 - 2)
                assert 0 <= r < NROW, (core, t, j, y, r)
                tbl[t, j] = r * ROWQ + (s + MARG)
                k, a_top, a_bot = by_chunk[j][0]
                tbl[t, NCHUNK + j] = k * 128
                alphas[:64, t * NTERM_COLS + j] = a_top
                alphas[64:, t * NTERM_COLS + j] = a_bot
                for extra in by_chunk[j][1:]:
                    acc_list.append((j,) + extra)
            else:
                tbl[t, j] = (t + 2) * ROWQ + MARG  # valid dummy offset
                tbl[t, NCHUNK + j] = 0
                # alphas stay 0 -> G chunk zero
        assert len(acc_list) <= NACC, (core, t, len(acc_list))
        for i in range(NACC):
            if i < len(acc_list):
                j, k, a_top, a_bot = acc_list[i]
                tbl[t, 2 * NCHUNK + i] = k * 128
                tbl[t, 2 * NCHUNK + NACC + i] = j * 128
                alphas[:64, t * NTERM_COLS + NCHUNK + i] = a_top
                alphas[64:, t * NTERM_COLS + NCHUNK + i] = a_bot
            else:
                tbl[t, 2 * NCHUNK + i] = 0
                tbl[t, 2 * NCHUNK + NACC + i] = 0  # alpha 0 -> adds zero to chunk 0
    return alphas, tbl.reshape(-1)


def _core_input_slab(xpc, core):
    """xpc: [B, C, IN_H, IN_W] padded channel-major input.
    Returns [B, C, NROW, ROWQ] f32 slab with circular x layout (q holds circ
    col q-1) and zero rows outside [0, IN_H)."""
    h0 = core * NH
    ys = np.arange(h0 - 2, h0 - 2 + NROW)
    valid = (ys >= 0) & (ys < IN_H)
    rows = np.zeros((B, C, NROW, IN_W), np.float32)
    rows[:, :, valid, :] = xpc[:, :, ys[valid], :]
    # circular layout: [col 257 | cols 0..257 | col 0]
    slab = np.concatenate([rows[..., -1:], rows, rows[..., :1]], axis=-1)
    assert slab.shape[-1] == ROWQ
    return np.ascontiguousarray(slab)


# ---------------------------------------------------------------- device code
def build_program():
    """Builds the uniform SPMD Bass program. Returns (nc, names) where names
    maps logical tensor roles to DRAM tensor names."""
    import concourse.bass as bass
    import concourse.mybir as mybir
    import concourse.tile as tile
    from concourse import bacc
    from concourse.bass import ds, ts

    f32 = mybir.dt.float32
    f32r = mybir.dt.float32r
    i32 = mybir.dt.int32

    nc = bacc.Bacc("TRN2", target_bir_lowering=False, debug=False)

    xs_d = nc.dram_tensor("xs", [B, C, NROW, ROWQ], f32r, kind="ExternalInput").ap()
    kt_d = nc.dram_tensor("kt", [KH * KW * C, F], f32r, kind="ExternalInput").ap()
    bias_d = nc.dram_tensor("bias", [F], f32, kind="ExternalInput").ap()
    al_d = nc.dram_tensor("alphas", [128, NH * NTERM_COLS], f32r, kind="ExternalInput").ap()
    tbl_d = nc.dram_tensor("tbl", [NH * (2 * NCHUNK + 2 * NACC)], i32, kind="ExternalInput").ap()
    out_d = nc.dram_tensor("out", [B, NH, F, W], f32, kind="ExternalOutput").ap()

    TBL_STRIDE = 2 * NCHUNK + 2 * NACC

    with tile.TileContext(nc) as tc:
        with (
            tc.tile_pool(name="const", bufs=1) as cpool,
            tc.tile_pool(name="gpool", bufs=3) as gpool,
            tc.tile_pool(name="pspool", bufs=2, space="PSUM") as pspool,
            tc.tile_pool(name="stpool", bufs=3) as stpool,
        ):
            # --- constant loads
            xst = cpool.tile([128, B, NROW * ROWQ], f32r)
            src_top = xs_d.rearrange("b c r q -> c b (r q)")
            nc.sync.dma_start(xst[0:64, :, :], src_top)
            flat_n = B * NROW * ROWQ
            src_bot = xs_d.rearrange("b c r q -> c (b r q)")[:, 1:flat_n]
            dst_bot = xst[64:128, :, :].rearrange("c b f -> c (b f)")[:, 0 : flat_n - 1]
            nc.sync.dma_start(dst_bot, src_bot)

            ktile = cpool.tile([128, KH * KW * 128], f32r)
            ksrc = kt_d.rearrange("(k c) f -> c (k f)", c=C)
            nc.sync.dma_start(ktile[0:64, :], ksrc)
            nc.sync.dma_start(ktile[64:128, :], ksrc)

            atile = cpool.tile([128, NH * NTERM_COLS], f32r)
            nc.sync.dma_start(atile[:, :], al_d)

            ttile = cpool.tile([1, NH * TBL_STRIDE], i32)
            nc.sync.dma_start(ttile[0:1, :], tbl_d.rearrange("n -> () n"))

            btile = cpool.tile([128, 1], f32)
            nc.sync.dma_start(btile[:, :], bias_d.rearrange("f -> f ()"))

            # --- registers (round-robin pools per engine)
            NREG_PE, NREG_DVE = 6, 8
            pe_regs = [nc.alloc_register(mybir.EngineType.PE) for _ in range(NREG_PE)]
            dve_regs = [nc.alloc_register(mybir.EngineType.DVE) for _ in range(NREG_DVE)]
            pe_sv = [nc.snap(r, donate=True) for r in pe_regs]
            dve_sv = [nc.snap(r, donate=True) for r in dve_regs]
            pe_i = dve_i = 0

            relu = mybir.ActivationFunctionType.Relu
            mult = mybir.AluOpType.mult
            add = mybir.AluOpType.add

            for t in range(NH):
                tb = t * TBL_STRIDE
                ac = t * NTERM_COLS
                gt = gpool.tile([128, NCHUNK * 128], f32r)

                # G init: one op per chunk
                for j in range(NCHUNK):
                    r = dve_i % NREG_DVE
                    dve_i += 1
                    nc.vector.reg_load(dve_regs[r], ttile[0:1, tb + NCHUNK + j : tb + NCHUNK + j + 1])
                    nc.vector.tensor_scalar(
                        out=gt[:, ts(j, 128)],
                        in0=ktile[:, ds(dve_sv[r], 128)],
                        scalar1=atile[:, ac + j : ac + j + 1],
                        scalar2=None,
                        op0=mult,
                    )
                # G accumulate
                for i in range(NACC):
                    rk = dve_i % NREG_DVE
                    dve_i += 1
                    rt = dve_i % NREG_DVE
                    dve_i += 1
                    nc.vector.reg_load(dve_regs[rk], ttile[0:1, tb + 2 * NCHUNK + i : tb + 2 * NCHUNK + i + 1])
                    nc.vector.reg_load(dve_regs[rt], ttile[0:1, tb + 2 * NCHUNK + NACC + i : tb + 2 * NCHUNK + NACC + i + 1])
                    nc.vector.scalar_tensor_tensor(
                        out=gt[:, ds(dve_sv[rt], 128)],
                        in0=ktile[:, ds(dve_sv[rk], 128)],
                        scalar=atile[:, ac + NCHUNK + i : ac + NCHUNK + i + 1],
                        in1=gt[:, ds(dve_sv[rt], 128)],
                        op0=mult,
                        op1=add,
                    )

                # main matmuls
                ps0 = pspool.tile([128, 2, 256], f32)
                ps1 = pspool.tile([128, 2, 256], f32)
                for j in range(NCHUNK):
                    r = pe_i % NREG_PE
                    pe_i += 1
                    nc.tensor.reg_load(pe_regs[r], ttile[0:1, tb + j : tb + j + 1])
                    for bp, pst in ((0, ps0), (1, ps1)):
                        nc.tensor.matmul(
                            pst[:, :, :],
                            lhsT=gt[:, ts(j, 128)],
                            rhs=xst[:, 2 * bp : 2 * bp + 2, ds(pe_sv[r], 256)],
                            start=(j == 0),
                            stop=(j == NCHUNK - 1),
                        )

                st = stpool.tile([128, B, 256], f32)
                nc.scalar.activation(st[:, 0:2, :], ps0[:, :, :], relu, bias=btile[:, 0:1])
                nc.scalar.activation(st[:, 2:4, :], ps1[:, :, :], relu, bias=btile[:, 0:1])
                for b in range(B):
                    nc.sync.dma_start(out_d[b, t], st[:, b, :])

    nc.compile()
    return nc


def make_in_maps(inputs, kernel, bias):
    chunks_all, terms_all = _build_chunk_tables()
    xp = np.pad(inputs.astype(np.float32), ((0, 0), (1, 1), (1, 1), (0, 0)))
    xpc = np.ascontiguousarray(xp.transpose(0, 3, 1, 2))  # [B, C, IN_H, IN_W]
    kt = np.ascontiguousarray(kernel.astype(np.float32))
    bs = np.ascontiguousarray(bias.astype(np.float32))
    in_maps = []
    for core in range(NCORE):
        alphas, tbl = _core_tables(core, chunks_all, terms_all)
        in_maps.append(
            {
                "xs": _core_input_slab(xpc, core),
                "kt": kt,
                "bias": bs,
                "alphas": alphas,
                "tbl": tbl,
            }
        )
    return in_maps


_PROGRAM_CACHE = {}


def kernel(inputs, kernel, bias):
    from concourse import bass_utils

    if "nc" not in _PROGRAM_CACHE:
        _PROGRAM_CACHE["nc"] = build_program()
    nc = _PROGRAM_CACHE["nc"]
    in_maps = make_in_maps(np.asarray(inputs), np.asarray(kernel), np.asarray(bias))
    res = bass_utils.run_bass_kernel_spmd(nc, in_maps, core_ids=list(range(NCORE)))
    # reassemble: per-core out [B, NH, F, W] -> [B, H, W, F]
    out = np.empty((B, H, W, F), np.float32)
    for core in range(NCORE):
        o = res.results[core]["out"]  # [B, NH, F, W]
        out[:, core * NH : (core + 1) * NH] = o.transpose(0, 1, 3, 2)
    return out


# revision 39
# speedup vs baseline: 1.0405x; 1.0405x over previous
"""DistortionConvLayer Trainium2 kernel (8-core SPMD, Bass/Tile).

Math: the distortion offsets depend only on (h, tap) and are compile-time
constants. Per (h, tap) the bilinear sample rows y0/y1 are fixed rows and the
x-coordinate is w + s with a constant integer shift s and constant fractional
part. Folding the four bilinear corner weights into the conv kernel gives

    out[b,h] = relu( sum_j  G[h,j]^T @ R[h,j]  + bias )            (F x W)

where chunk j has a (row y, shift s) pair,
    R[h,j] = [ Xc[y, w+s] ; Xc[y, w+s+1] ]   (128 x W, c-major, circular x)
    G[h,j] = sum over taps (k, yrow) hitting (y, s):
                [ wy*wx0 * K_k ; wy*wx1 * K_k ]   (128 x F)

G depends only on the runtime conv kernel (a host-side weight repack), so all
G tables are precomputed in numpy and shipped per core; the device program is
pure fp16 matmuls (N=512, two batch images per matmul) accumulating in fp32
PSUM, a fused ReLU+bias on the scalar engine, and DMA.

Sharding: H is split into 8 contiguous blocks of 16 rows; each core processes
all 4 batch images for its rows. One uniform SPMD program serves all cores:
slot lists vary per local row index t (compile-time) but are shared across
cores (union over cores); per-core variation lives entirely in input data
(row slab with halo, G tables).
"""

import numpy as np

# problem dims (hardcoded per spec)
B, H, W, C, F = 4, 128, 256, 64, 128
KH = KW = 3
IN_H, IN_W = H + 2, W + 2
NCORE = 8
NH = H // NCORE            # h rows per core
NROW = NH + 6              # input rows per core: [h0-2, h0+NH+4)
MARG = 1                   # left margin in circular row layout
ROWQ = 260                 # stored row width: q in [0,260) holds circ col (q-1)
NCHUNK = 14                # padded chunks per h
NACC = 8                   # padded accumulate terms per h
NTERM_COLS = NCHUNK + NACC # alpha columns per h


# ---------------------------------------------------------------- host tables
def _make_offset(h, w, dilation=1.0, skydome=True):
    pi = np.pi
    unit_w = 2.0 * pi / w
    unit_h = pi / (2.0 * h) if skydome else pi / h
    rho = np.tan(unit_w) * dilation
    v = np.array([0.0, 1.0, 0.0])
    r_grid = np.array(
        [[1, -1], [1, 0], [1, 1], [0, -1], [0, 0], [0, 1], [-1, -1], [-1, 0], [-1, 1]],
        dtype=np.float64,
    )
    xc = int(w * 0.5)
    theta = (xc - 0.5 * w) * unit_w
    y = np.arange(h, dtype=np.float64)
    phi = (h - y) * unit_h if skydome else (h * 0.5 - y) * unit_h
    p_u = np.stack(
        [np.cos(phi) * np.cos(theta), np.sin(phi), np.cos(phi) * np.sin(theta)], axis=-1
    )
    t_x = np.cross(np.broadcast_to(v, p_u.shape), p_u)
    t_y = np.cross(p_u, t_x)
    r_sphere = rho * (
        r_grid[None, :, 0, None] * t_x[:, None, :]
        + r_grid[None, :, 1, None] * t_y[:, None, :]
    )
    p_ur = p_u[:, None, :] + r_sphere
    ux, uy, uz = p_ur[..., 0], p_ur[..., 1], p_ur[..., 2]
    base = np.arctan2(uz, ux)
    theta_r = np.where(
        ux > 0,
        base,
        np.where(
            ux < 0,
            np.where(uz >= 0, base + pi, base - pi),
            np.where(uz > 0, pi * 0.5, -pi * 0.5),
        ),
    )
    phi_r = np.arcsin(uy)
    x_r = (theta_r / pi + 1.0) * 0.5 * w
    y_r = (1.0 - 2.0 * phi_r / pi) * h if skydome else (0.5 - phi_r / pi) * h
    k = np.stack([x_r, y_r], axis=-1)
    off = k - k[:, 4:5, :]
    return off.astype(np.float32)  # [h, 9, 2]


def _build_chunk_tables():
    """Per-h chunk decomposition.

    Returns (chunks, terms): chunks[h] = [(y, s)], terms[h] = list of
    (tap k, chunk idx, a_top, a_bot) with 18 entries.
    """
    off = _make_offset(H, W)
    chunks_all, terms_all = [], []
    for h in range(H):
        ids, chunks, terms = {}, [], []
        for k in range(KH * KW):
            dy, dx = k // 3, k % 3
            cy, cx = np.float32(off[h, k, 0]), np.float32(off[h, k, 1])
            yv = float(np.float32(h + dy) + cy)
            yv = min(max(yv, 0.0), float(IN_H - 1))
            y0 = min(max(int(np.floor(yv)), 0), IN_H - 1)
            y1 = min(y0 + 1, IN_H - 1)
            wy0, wy1 = float(y1 - yv), float(yv - y0)
            s = dx + int(np.floor(cx))
            fx = float(dx + cx - np.floor(cx + dx))
            wx0, wx1 = 1.0 - fx, fx
            for yy, wy in ((y0, wy0), (y1, wy1)):
                if wy == 0.0:
                    continue
                key = (yy, s)
                if key not in ids:
                    ids[key] = len(chunks)
                    chunks.append(key)
                terms.append((k, ids[key], wy * wx0, wy * wx1))
        assert len(chunks) <= NCHUNK
        chunks_all.append(chunks)
        terms_all.append(terms)
    return chunks_all, terms_all


def _corner_sets(chunks_all, terms_all):
    """Per h: list of (rho, sigma, weight, tap) corner contributions."""
    corners_all = []
    for h in range(H):
        chunks, terms = chunks_all[h], terms_all[h]
        cs = []
        for (k, j, a_top, a_bot) in terms:
            y, sg = chunks[j]
            if a_top != 0.0:
                cs.append((y - h, sg, a_top, k))
            if a_bot != 0.0:
                cs.append((y - h, sg + 1, a_bot, k))
        corners_all.append(cs)
    return corners_all


def _build_static_plan(chunks_all, terms_all):
    """Global plan: per t, a greedy set cover of the 8-core union of corner
    needs by (rho, s) chunks (top half covers sigma=s, bottom sigma=s+1)."""
    corners_all = _corner_sets(chunks_all, terms_all)
    slots_all = []
    for t in range(NH):
        need = set()
        for p in range(NCORE):
            need |= {(r, sg) for (r, sg, _w, _k) in corners_all[p * NH + t]}
        slots = []
        needc = set(need)
        while needc:
            best, bc = None, -1
            for (r, sg) in sorted(needc):
                for cand in ((r, sg), (r, sg - 1)):
                    cov = len({(cand[0], cand[1]), (cand[0], cand[1] + 1)} & needc)
                    if cov > bc:
                        bc, best = cov, cand
            slots.append(best)
            needc -= {(best[0], best[1]), (best[0], best[1] + 1)}
        slots.sort()
        # safety: every needed corner covered
        covered = set()
        for (r, sg) in slots:
            covered |= {(r, sg), (r, sg + 1)}
        assert need <= covered
        slots_all.append(slots)
    return corners_all, slots_all


def _core_g_tables(core, corners_all, slots_all, kernel):
    """Host-computed per-core G tables [128, sum_t nslot(t)*128] fp16.
    Each corner contribution is assigned to one covering slot (top half if
    slot s == sigma, else bottom half of slot s == sigma-1)."""
    totg = sum(len(sl) for sl in slots_all)
    g = np.zeros((128, totg * 128), np.float32)
    goff = 0
    for t in range(NH):
        slots = slots_all[t]
        sid = {key: i for i, key in enumerate(slots)}
        for (r, sg, w, k) in corners_all[core * NH + t]:
            Kk = kernel[k * C : (k + 1) * C, :]
            if (r, sg) in sid:
                i, half = sid[(r, sg)], 0
            else:
                i, half = sid[(r, sg - 1)], 1
            lo = 64 * half
            g[lo : lo + 64, (goff + i) * 128 : (goff + i + 1) * 128] += np.float32(w) * Kk
        goff += len(slots)
    return np.ascontiguousarray(g.astype(np.float16))


def _core_input_slab(xpc, core):
    """xpc: [B, C, IN_H, IN_W] padded channel-major input.
    Returns [B, C, NROW, ROWQ] f32 slab with circular x layout (q holds circ
    col q-1) and zero rows outside [0, IN_H)."""
    h0 = core * NH
    ys = np.arange(h0 - 2, h0 - 2 + NROW)
    valid = (ys >= 0) & (ys < IN_H)
    rows = np.zeros((B, C, NROW, IN_W), np.float32)
    rows[:, :, valid, :] = xpc[:, :, ys[valid], :]
    # circular layout: [col 257 | cols 0..257 | col 0]
    slab = np.concatenate([rows[..., -1:], rows, rows[..., :1]], axis=-1)
    assert slab.shape[-1] == ROWQ
    return np.ascontiguousarray(slab)


# ---------------------------------------------------------------- device code
def build_program():
    """Uniform SPMD Bass program: pure matmul + relu (G precomputed on host)."""
    import concourse.mybir as mybir
    import concourse.tile as tile
    from concourse import bacc
    from concourse.bass import ts

    f32 = mybir.dt.float32
    f16 = mybir.dt.float16

    chunks_all, terms_all = _build_chunk_tables()
    corners_all, slots_all = _build_static_plan(chunks_all, terms_all)
    totg = sum(len(sl) for sl in slots_all)

    nc = bacc.Bacc("TRN2", target_bir_lowering=False, debug=False)

    xs_d = nc.dram_tensor("xs", [B, C, NROW, ROWQ], f16, kind="ExternalInput").ap()
    g_d = nc.dram_tensor("g", [128, totg * 128], f16, kind="ExternalInput").ap()
    bias_d = nc.dram_tensor("bias", [F], f32, kind="ExternalInput").ap()
    out_d = nc.dram_tensor("out", [B, NH, F, W], f32, kind="ExternalOutput").ap()

    with tile.TileContext(nc) as tc:
        with (
            tc.tile_pool(name="const", bufs=1) as cpool,
            tc.tile_pool(name="pspool", bufs=4, space="PSUM") as pspool,
            tc.tile_pool(name="stpool", bufs=3) as stpool,
        ):
            xst = cpool.tile([128, B, NROW * ROWQ], f16)
            gtile = cpool.tile([128, totg * 128], f16)
            btile = cpool.tile([128, 1], f32)
            src_top = xs_d.rearrange("b c r q -> c b (r q)")
            flat_n = NROW * ROWQ

            nc.sync.dma_start(btile[:, :], bias_d.rearrange("f -> f ()"))

            # interleave G and row-chunk loads in consumption order so the
            # first matmuls unblock within a few microseconds
            g_bounds = [0]
            for sl in slots_all:
                g_bounds.append(g_bounds[-1] + len(sl) * 128)

            dma_engs = [nc.sync, nc.gpsimd]
            _ei = [0]

            def _eng():
                e = dma_engs[_ei[0] % len(dma_engs)]
                _ei[0] += 1
                return e

            def emit_g(t):
                _eng().dma_start(
                    gtile[:, g_bounds[t] : g_bounds[t + 1]],
                    g_d[:, g_bounds[t] : g_bounds[t + 1]],
                )

            def emit_rows(r0, r1, bs=(0, 1, 2, 3), top=True):
                c0, c1 = r0 * ROWQ, r1 * ROWQ
                for b in bs:
                    if top:
                        _eng().dma_start(xst[0:64, b, c0:c1], src_top[:, b, c0:c1])
                    c1b = min(c1 + 1, flat_n)
                    _eng().dma_start(
                        xst[64:128, b, c0 : c1b - 1], src_top[:, b, c0 + 1 : c1b]
                    )

            row_chunks = [(0, 7), (7, 10), (10, 13), (13, 16), (16, 19), (19, NROW)]
            # first-needed pieces, smallest first: G0 front slots + first rows of b0/b1
            nc.scalar.dma_start(gtile[:, 0 : 4 * 128], g_d[:, 0 : 4 * 128])
            # first chunk's bottom half straight from HBM (no copy-chain hop)
            for b in (0, 1):
                nc.sync.dma_start(xst[0:64, b, 0 : 2 * ROWQ], src_top[:, b, 0 : 2 * ROWQ])
                nc.gpsimd.dma_start(
                    xst[64:128, b, 0 : 2 * ROWQ - 1], src_top[:, b, 1 : 2 * ROWQ]
                )
            nc.scalar.dma_start(gtile[:, 4 * 128 : g_bounds[1]], g_d[:, 4 * 128 : g_bounds[1]])
            emit_rows(0, 2, bs=(2, 3))
            emit_rows(2, 4)
            emit_rows(4, 7)
            emit_g(1)
            emit_g(2)
            emit_rows(*row_chunks[1])
            emit_g(3)
            emit_g(4)
            emit_rows(*row_chunks[2])
            emit_g(5)
            emit_g(6)
            emit_g(7)
            emit_g(8)
            emit_rows(*row_chunks[3])
            emit_g(9)
            emit_g(10)
            emit_g(11)
            emit_g(12)
            emit_rows(*row_chunks[4])
            emit_g(13)
            emit_g(14)
            emit_g(15)
            emit_rows(*row_chunks[5])

            relu = mybir.ActivationFunctionType.Relu

            goff = 0
            for t in range(NH):
                slots = slots_all[t]
                nslot = len(slots)
                ps0 = pspool.tile([128, 2, 256], f32)
                ps1 = pspool.tile([128, 2, 256], f32)
                for bp, pst in ((0, ps0), (1, ps1)):
                    for j, (rho, sig) in enumerate(slots):
                        off = (t + 2 + rho) * ROWQ + (sig + MARG)
                        nc.tensor.matmul(
                            pst[:, :, :],
                            lhsT=gtile[:, ts(goff + j, 128)],
                            rhs=xst[:, 2 * bp : 2 * bp + 2, off : off + 256],
                            start=(j == 0),
                            stop=(j == nslot - 1),
                        )
                goff += nslot
                st = stpool.tile([128, B, 256], f32)
                nc.scalar.activation(st[:, 0:2, :], ps0[:, :, :], relu, bias=btile[:, 0:1])
                nc.scalar.activation(st[:, 2:4, :], ps1[:, :, :], relu, bias=btile[:, 0:1])
                for b in range(B):
                    nc.sync.dma_start(out_d[b, t], st[:, b, :])
            assert goff == totg

    nc.compile()
    return nc


def make_in_maps(inputs, kernel, bias):
    chunks_all, terms_all = _build_chunk_tables()
    corners_all, slots_all = _build_static_plan(chunks_all, terms_all)
    xp = np.pad(inputs.astype(np.float32), ((0, 0), (1, 1), (1, 1), (0, 0)))
    xpc = np.ascontiguousarray(xp.transpose(0, 3, 1, 2))  # [B, C, IN_H, IN_W]
    kf = np.asarray(kernel, np.float32)
    bs = np.ascontiguousarray(bias.astype(np.float32))
    in_maps = []
    for core in range(NCORE):
        in_maps.append(
            {
                "xs": _core_input_slab(xpc, core).astype(np.float16),
                "g": _core_g_tables(core, corners_all, slots_all, kf),
                "bias": bs,
            }
        )
    return in_maps


_PROGRAM_CACHE = {}


def kernel(inputs, kernel, bias):
    from concourse import bass_utils

    if "nc" not in _PROGRAM_CACHE:
        _PROGRAM_CACHE["nc"] = build_program()
    nc = _PROGRAM_CACHE["nc"]
    in_maps = make_in_maps(np.asarray(inputs), np.asarray(kernel), np.asarray(bias))
    res = bass_utils.run_bass_kernel_spmd(nc, in_maps, core_ids=list(range(NCORE)))
    out = np.empty((B, H, W, F), np.float32)
    for core in range(NCORE):
        o = res.results[core]["out"]  # [B, NH, F, W]
        out[:, core * NH : (core + 1) * NH] = o.transpose(0, 1, 3, 2)
    return out
